# revision 1
# baseline (speedup 1.0000x reference)
"""2-layer GCN (100k nodes, 3.2M edges) on 8 Trainium2 NeuronCores.

Strategy (graph/data parallel, per the node-partition + halo-exchange hint):
  - Nodes are range-partitioned across the 8 cores (12500 each + 44 dummies
    -> 12544 = 98*128 positions per core).
  - GCN algebra: out = D^-1/2 A_hat D^-1/2 (H W).  We pre-scale each node's
    transformed features by dinv, segment-sum over in-edges, and post-scale
    by dinv; for layer 2 we aggregate first and apply W2 after (linearity),
    so both layers aggregate 16-dim features.
  - Per layer, each core computes its shard of the (scaled) feature table,
    the shards are AllGather'd (the halo exchange: feature-major [16, 12544]
    f32 per core -> [128, 12544] global table resident in SBUF).
  - Aggregation: edges are grouped by the core that owns their SRC (= the
    16-partition GPSIMD group holding that core's table slice).  Each group
    gathers its edges' source features with the ap_gather ucode; per-node
    slot counts are padded to a uniform width per 128-node block (nodes
    degree-sorted so padding is small); a DVE segmented reduce produces
    per-group partial sums; a PE matmul against a replicated selector
    (layer 1) or replicated W2 (layer 2) sums across the 8 groups.

All floating-point arithmetic (matmuls, degree->rsqrt, aggregation, bias,
relu, log_softmax) runs on device.  The host only restructures integers
(edge lists -> per-block index tensors) and permutes/relayouts tensors.
"""

import numpy as np

import concourse.bass as bass
import concourse.bacc as bacc
import concourse.mybir as mybir
import concourse.tile as tile
from concourse.bass_utils import run_bass_kernel_spmd

N_NODES = 100000
N_FEAT = 512
HIDDEN = 16
N_CLASSES = 64
NCORES = 8
NPC_REAL = 12500          # real nodes per core
NPC = 12544               # padded positions per core (98 * 128)
NBLK = NPC // 128         # 98 blocks of 128 nodes
SB = 4                    # blocks per super-block (ap_gather/reduce batch)
DUMMY_COL = NPC - 1       # every core's last position is a dummy (zero) node

_cache = {}


# ----------------------------------------------------------------------------
# host-side graph restructuring (integer work only)
# ----------------------------------------------------------------------------

def _preprocess(edge_index):
    src = edge_index[0].astype(np.int64)
    dst = edge_index[1].astype(np.int64)

    # in-degree INCLUDES the self-loop; but self-loop edges are handled
    # locally (shard add), not gathered, so they are excluded from the slots
    deg = np.bincount(dst, minlength=N_NODES) + 1

    owner_src = src // NPC_REAL

    m = np.bincount(dst * 8 + owner_src, minlength=N_NODES * 8).reshape(
        N_NODES, 8
    )                                                            # per-group counts
    dtil = m.max(axis=1)                                         # slots per node

    # per-core permutation: sort local nodes by dtil desc; dummies (dtil=-1) last
    order = np.empty((NCORES, NPC), dtype=np.int64)   # position -> local node id
    rank = np.empty(N_NODES, dtype=np.int64)          # global node -> position
    for c in range(NCORES):
        lo = c * NPC_REAL
        d_loc = np.concatenate(
            [dtil[lo : lo + NPC_REAL], np.full(NPC - NPC_REAL, -1, np.int64)]
        )
        o = np.argsort(-d_loc, kind="stable")
        order[c] = o
        inv = np.empty(NPC, dtype=np.int64)
        inv[o] = np.arange(NPC)
        rank[lo : lo + NPC_REAL] = inv[:NPC_REAL]

    # block widths, unified across cores; grouped into super-blocks
    # dtil at position (c, pos): for a block the max is at its first position
    dtil_pos = np.zeros((NCORES, NPC), dtype=np.int64)
    for c in range(NCORES):
        lo = c * NPC_REAL
        real = order[c] < NPC_REAL
        dtil_pos[c][real] = dtil[lo + order[c][real]]
    # adaptive super-blocks: pack consecutive blocks while nodes*width <= cap
    # (amortizes the ~1us fixed cost per ap_gather instruction)
    blk_D = np.zeros(NBLK, dtype=np.int64)
    for b in range(NBLK):
        blk_D[b] = max(1, dtil_pos[:, b * 128 : (b + 1) * 128].max())
    NI_CAP = max(4096, int(128 * blk_D.max()))
    supers = []  # (b0, nblk, D)
    b = 0
    while b < NBLK:
        D = blk_D[b]
        nb = 1
        while (
            b + nb < NBLK
            and nb < SB
            and (nb + 1) * 128 * max(D, blk_D[b + nb]) <= NI_CAP
        ):
            D = max(D, blk_D[b + nb])
            nb += 1
        supers.append((b, nb, int(D)))
        b += nb
    n_super = len(supers)
    sup_of_blk = np.zeros(NBLK, dtype=np.int64)
    for si, (b0, nb, D) in enumerate(supers):
        sup_of_blk[b0 : b0 + nb] = si
    DSUP = np.array([D for (_, _, D) in supers], dtype=np.int64)
    sup_b0 = np.array([b0 for (b0, _, _) in supers], dtype=np.int64)
    num_idxs = np.array([nb * 128 * D for (_, nb, D) in supers], dtype=np.int64)
    colbase = np.zeros(n_super + 1, dtype=np.int64)
    colbase[1:] = np.cumsum(num_idxs // 16)
    IDXCOLS = int(colbase[-1])

    # per-edge slot assignment (vectorized)
    key = dst * 8 + owner_src
    perm = np.argsort(key, kind="stable")
    key_s = key[perm]
    src_s = src[perm]
    starts = np.zeros(N_NODES * 8 + 1, dtype=np.int64)
    starts[1:] = np.cumsum(m.ravel())
    j_within = np.arange(len(src_s), dtype=np.int64) - starts[key_s]

    dst_s = key_s // 8
    g_s = key_s % 8
    c_s = dst_s // NPC_REAL
    pos_s = rank[dst_s]                               # position within core
    blk_s = pos_s // 128
    i_s = pos_s % 128
    sup_s = sup_of_blk[blk_s]
    node_in_sup = (blk_s - sup_b0[sup_s]) * 128 + i_s
    e_col = node_in_sup * DSUP[sup_s] + j_within      # column within instruction
    part = 16 * g_s + (e_col % 16)
    col = colbase[sup_s] + e_col // 16
    val = rank[src_s]                                 # table column of the source

    idx_all = np.full((NCORES, 128, IDXCOLS), DUMMY_COL, dtype=np.int16)
    idx_all[c_s, part, col] = val.astype(np.int16)

    # per-core degree tensors in (partition, block) layout
    deg_pb = np.zeros((NCORES, 128, NBLK), dtype=np.int32)
    for c in range(NCORES):
        lo = c * NPC_REAL
        real = order[c] < NPC_REAL
        d = np.zeros(NPC, dtype=np.int32)
        d[real] = deg[lo + order[c][real]].astype(np.int32)
        deg_pb[c] = d.reshape(NBLK, 128).T            # pos = b*128 + p
    # deg repeated 16x along free dim for batched layer-1 scaling
    deg_rep = np.repeat(deg_pb, HIDDEN, axis=2).reshape(NCORES, 128, NBLK * HIDDEN)
    # note: repeat on axis=2 of [C,128,NBLK] gives [C,128,NBLK*16] with each
    # block's degree contiguous 16 wide -- matches q layout [128, (b f)]

    return {
        "order": order,
        "idx_all": idx_all,
        "deg_pb": deg_pb,
        "deg_rep": deg_rep,
        "supers": supers,
        "num_idxs": num_idxs,
        "colbase": colbase,
        "IDXCOLS": IDXCOLS,
        "NI_CAP": NI_CAP,
    }


# ----------------------------------------------------------------------------
# device program
# ----------------------------------------------------------------------------

def _build_program(meta):
    supers = meta["supers"]
    num_idxs = meta["num_idxs"]
    colbase = meta["colbase"]
    IDXCOLS = meta["IDXCOLS"]
    NI_CAP = meta["NI_CAP"]
    n_phA = (NBLK + SB - 1) // SB      # phase-A block groups (fixed SB)
    f32 = mybir.dt.float32

    nc = bacc.Bacc(
        "TRN2", target_bir_lowering=False, debug=False, num_devices=NCORES
    )
    xT = nc.declare_dram_parameter("xT", [N_FEAT, NPC], f32, isOutput=False)
    idx_in = nc.declare_dram_parameter(
        "idx_in", [128, IDXCOLS], mybir.dt.int16, isOutput=False
    )
    degrep_in = nc.declare_dram_parameter(
        "degrep_in", [128, NBLK * HIDDEN], mybir.dt.int32, isOutput=False
    )
    W1r_in = nc.declare_dram_parameter("W1r", [128, 64], f32, isOutput=False)
    b1r_in = nc.declare_dram_parameter("b1r", [128, SB * HIDDEN], f32, isOutput=False)
    E8I_in = nc.declare_dram_parameter("E8I", [128, HIDDEN], f32, isOutput=False)
    W2r_in = nc.declare_dram_parameter("W2r", [128, N_CLASSES], f32, isOutput=False)
    b2r_in = nc.declare_dram_parameter(
        "b2r", [128, SB * N_CLASSES], f32, isOutput=False
    )
    ident_in = nc.declare_dram_parameter("ident", [128, 128], f32, isOutput=False)
    dmask_in = nc.declare_dram_parameter("dmask", [128, 1], f32, isOutput=False)
    out_d = nc.declare_dram_parameter("out", [NBLK, 128, N_CLASSES], f32, isOutput=True)

    q1d = nc.dram_tensor("q1d", [16, NPC], f32)
    q2d = nc.dram_tensor("q2d", [16, NPC], f32)
    tab1d = nc.dram_tensor("tab1d", [128, NPC], f32, addr_space="Shared")
    tab2d = nc.dram_tensor("tab2d", [128, NPC], f32, addr_space="Shared")

    rg = [list(range(NCORES))]

    with tile.TileContext(nc) as tc:
        with (
            tc.tile_pool(name="const", bufs=1) as cp,
            tc.tile_pool(name="xt", bufs=2) as xp,
            tc.tile_pool(name="msg", bufs=2) as mp,
            tc.tile_pool(name="work", bufs=3) as wp,
            tc.tile_pool(name="shard", bufs=1) as sp,
            tc.tile_pool(name="tab", bufs=1) as tp,
            tc.tile_pool(name="ps", bufs=2, space="PSUM") as pp,
            tc.tile_pool(name="psT", bufs=2, space="PSUM") as ppT,
            tc.tile_pool(name="psO", bufs=2, space="PSUM") as ppO,
        ):
            # ---- constants -------------------------------------------------
            W1r = cp.tile([128, 64], f32)
            nc.sync.dma_start(out=W1r[:], in_=W1r_in[:])
            b1r = cp.tile([128, SB * HIDDEN], f32)
            nc.sync.dma_start(out=b1r[:], in_=b1r_in[:])
            E8I = cp.tile([128, HIDDEN], f32)
            nc.sync.dma_start(out=E8I[:], in_=E8I_in[:])
            W2r = cp.tile([128, N_CLASSES], f32)
            nc.sync.dma_start(out=W2r[:], in_=W2r_in[:])
            b2r = cp.tile([128, SB * N_CLASSES], f32)
            nc.sync.dma_start(out=b2r[:], in_=b2r_in[:])
            ident = cp.tile([128, 128], f32)
            nc.sync.dma_start(out=ident[:], in_=ident_in[:])
            dmask = cp.tile([128, 1], f32)
            nc.sync.dma_start(out=dmask[:], in_=dmask_in[:])
            idx_sb = cp.tile([128, IDXCOLS], mybir.dt.int16)
            nc.sync.dma_start(out=idx_sb[:], in_=idx_in[:])

            # dinv (repeated 16x per block): rsqrt(max(deg,1)) on device
            degrep = cp.tile([128, NBLK * HIDDEN], mybir.dt.int32)
            nc.sync.dma_start(out=degrep[:], in_=degrep_in[:])
            dinvr = cp.tile([128, NBLK * HIDDEN], f32)
            nc.vector.tensor_copy(out=dinvr[:], in_=degrep[:])
            nc.vector.tensor_scalar_max(out=dinvr[:], in0=dinvr[:], scalar1=1.0)
            nc.vector.reciprocal(out=dinvr[:], in_=dinvr[:])
            nc.scalar.activation(
                out=dinvr[:], in_=dinvr[:], func=mybir.ActivationFunctionType.Sqrt
            )

            shard = sp.tile([16, NPC], f32)   # feat-major shard (reused q1/q2)
            table = tp.tile([128, NPC], f32)  # gathered global table

            def post_to_shard(qa4, b0, nblk_s):
                """transpose node-major [128, nblk_s*16] -> shard strips."""
                for j in range(nblk_s):
                    b = b0 + j
                    psT = ppT.tile([HIDDEN, 128], f32, tag="psT")
                    nc.tensor.transpose(
                        out=psT[:],
                        in_=qa4[:, j * HIDDEN : (j + 1) * HIDDEN],
                        identity=ident[:],
                    )
                    nc.vector.tensor_copy(
                        out=shard[:, b * 128 : (b + 1) * 128], in_=psT[:]
                    )

            # ---- phase A: q1 = (x @ W1) * dinv, feat-major shard -----------
            for s in range(n_phA):
                b0 = s * SB
                nblk_s = min(SB, NBLK - b0)
                w = nblk_s * 128
                xts = []
                for kc in range(4):
                    xt = xp.tile([128, SB * 128], f32, tag=f"xt{kc}")
                    nc.sync.dma_start(
                        out=xt[:, :w],
                        in_=xT[kc * 128 : (kc + 1) * 128, b0 * 128 : b0 * 128 + w],
                    )
                    xts.append(xt)
                qa4 = wp.tile([128, SB * HIDDEN], f32, tag="qa4")
                for j in range(nblk_s):
                    b = b0 + j
                    psA = pp.tile([128, HIDDEN], f32, tag="psA")
                    for kc in range(4):
                        nc.tensor.matmul(
                            out=psA[:],
                            lhsT=xts[kc][:, j * 128 : (j + 1) * 128],
                            rhs=W1r[:, kc * HIDDEN : (kc + 1) * HIDDEN],
                            start=(kc == 0),
                            stop=(kc == 3),
                        )
                    nc.vector.tensor_tensor(
                        out=qa4[:, j * HIDDEN : (j + 1) * HIDDEN],
                        in0=psA[:],
                        in1=dinvr[:, b * HIDDEN : (b + 1) * HIDDEN],
                        op=mybir.AluOpType.mult,
                    )
                post_to_shard(qa4, b0, nblk_s)
            nc.sync.dma_start(out=q1d[:], in_=shard[:])

            # ---- allgather 1 + table load ---------------------------------
            nc.gpsimd.collective_compute(
                "AllGather",
                mybir.AluOpType.bypass,
                replica_groups=rg,
                ins=[q1d[:]],
                outs=[tab1d[:]],
            )
            nc.sync.dma_start(out=table[:], in_=tab1d[:])

            # ---- aggregation helper ---------------------------------------
            def aggregate(s):
                """gather + segmented reduce; returns [128, nodes] partials."""
                b0, nblk_s, D = supers[s]
                nodes = nblk_s * 128
                ni = int(num_idxs[s])
                msg = mp.tile([128, NI_CAP], f32, tag="msg")
                nc.gpsimd.ap_gather(
                    out_ap=msg[:, :ni],
                    in_ap=table[:],
                    idxs_ap=idx_sb[:, int(colbase[s]) : int(colbase[s + 1])],
                    channels=128,
                    num_elems=NPC,
                    d=1,
                    num_idxs=ni,
                )
                part = wp.tile([128, SB * 128], f32, tag="part")
                nc.vector.tensor_reduce(
                    out=part[:, :nodes],
                    in_=msg[:, :ni].rearrange("p (n d) -> p n d", d=D),
                    axis=mybir.AxisListType.X,
                    op=mybir.AluOpType.add,
                )
                # self-loop contribution: q[n] is resident in the local shard;
                # add it into one group's partial rows (the cross-group matmul
                # sums over all 8 groups, so any one group works)
                nc.vector.tensor_tensor(
                    out=part[0:16, :nodes],
                    in0=part[0:16, :nodes],
                    in1=shard[:, b0 * 128 : b0 * 128 + nodes],
                    op=mybir.AluOpType.add,
                )
                return part, b0, nblk_s

            # ---- layer 1 aggregation -> q2 shard --------------------------
            for s in range(len(supers)):
                part, b0, nblk_s = aggregate(s)
                psX = pp.tile([128, SB * HIDDEN], f32, tag="psA")
                for j in range(nblk_s):
                    nc.tensor.matmul(
                        out=psX[:, j * HIDDEN : (j + 1) * HIDDEN],
                        lhsT=part[:, j * 128 : (j + 1) * 128],
                        rhs=E8I[:],
                        start=True,
                        stop=True,
                    )
                qa4 = wp.tile([128, SB * HIDDEN], f32, tag="qa4")
                dslice = dinvr[:, b0 * HIDDEN : b0 * HIDDEN + nblk_s * HIDDEN]
                ql = qa4[:, : nblk_s * HIDDEN]
                nc.vector.tensor_tensor(
                    out=ql, in0=psX[:, : nblk_s * HIDDEN], in1=dslice,
                    op=mybir.AluOpType.mult,
                )
                nc.vector.tensor_tensor(
                    out=ql, in0=ql, in1=b1r[:, : nblk_s * HIDDEN],
                    op=mybir.AluOpType.add,
                )
                nc.vector.tensor_scalar_max(out=ql, in0=ql, scalar1=0.0)
                nc.vector.tensor_tensor(
                    out=ql, in0=ql, in1=dslice, op=mybir.AluOpType.mult
                )
                if b0 + nblk_s == NBLK:  # kill dummy nodes (last block tail)
                    sl = qa4[:, (nblk_s - 1) * HIDDEN : nblk_s * HIDDEN]
                    nc.vector.tensor_scalar_mul(out=sl, in0=sl, scalar1=dmask[:, :1])
                post_to_shard(qa4, b0, nblk_s)
            nc.sync.dma_start(out=q2d[:], in_=shard[:])

            # ---- allgather 2 + table load ---------------------------------
            nc.gpsimd.collective_compute(
                "AllGather",
                mybir.AluOpType.bypass,
                replica_groups=rg,
                ins=[q2d[:]],
                outs=[tab2d[:]],
            )
            nc.sync.dma_start(out=table[:], in_=tab2d[:])

            # ---- layer 2 aggregation -> logits -> log_softmax -------------
            for s in range(len(supers)):
                part, b0, nblk_s = aggregate(s)
                psO = ppO.tile([128, SB * N_CLASSES], f32, tag="psO")
                for j in range(nblk_s):
                    nc.tensor.matmul(
                        out=psO[:, j * N_CLASSES : (j + 1) * N_CLASSES],
                        lhsT=part[:, j * 128 : (j + 1) * 128],
                        rhs=W2r[:],
                        start=True,
                        stop=True,
                    )
                z4 = wp.tile([128, SB * N_CLASSES], f32, tag="z4")
                for j in range(nblk_s):
                    b = b0 + j
                    nc.vector.tensor_scalar_mul(
                        out=z4[:, j * N_CLASSES : (j + 1) * N_CLASSES],
                        in0=psO[:, j * N_CLASSES : (j + 1) * N_CLASSES],
                        scalar1=dinvr[:, b * HIDDEN : b * HIDDEN + 1],
                    )
                zl = z4[:, : nblk_s * N_CLASSES]
                nc.vector.tensor_tensor(
                    out=zl, in0=zl, in1=b2r[:, : nblk_s * N_CLASSES],
                    op=mybir.AluOpType.add,
                )
                negm = wp.tile([128, SB], f32, tag="negm")
                nc.vector.tensor_reduce(
                    out=negm[:, :nblk_s],
                    in_=zl.rearrange("p (n c) -> p n c", c=N_CLASSES),
                    axis=mybir.AxisListType.X,
                    op=mybir.AluOpType.max,
                    negate=True,
                )
                e4 = wp.tile([128, SB * N_CLASSES], f32, tag="e4")
                ssum = wp.tile([128, SB], f32, tag="ssum")
                for j in range(nblk_s):
                    nc.scalar.activation(
                        out=e4[:, j * N_CLASSES : (j + 1) * N_CLASSES],
                        in_=z4[:, j * N_CLASSES : (j + 1) * N_CLASSES],
                        func=mybir.ActivationFunctionType.Exp,
                        bias=negm[:, j : j + 1],
                        scale=1.0,
                        accum_out=ssum[:, j : j + 1],
                    )
                ls = wp.tile([128, SB], f32, tag="ls")
                nc.scalar.activation(
                    out=ls[:, :nblk_s],
                    in_=ssum[:, :nblk_s],
                    func=mybir.ActivationFunctionType.Ln,
                )
                o4 = wp.tile([128, SB * N_CLASSES], f32, tag="o4")
                for j in range(nblk_s):
                    nc.vector.tensor_scalar(
                        out=o4[:, j * N_CLASSES : (j + 1) * N_CLASSES],
                        in0=z4[:, j * N_CLASSES : (j + 1) * N_CLASSES],
                        scalar1=negm[:, j : j + 1],
                        scalar2=ls[:, j : j + 1],
                        op0=mybir.AluOpType.add,
                        op1=mybir.AluOpType.subtract,
                    )
                for j in range(nblk_s):
                    nc.sync.dma_start(
                        out=out_d[b0 + j],
                        in_=o4[:, j * N_CLASSES : (j + 1) * N_CLASSES],
                    )

    nc.finalize()
    return nc


# ----------------------------------------------------------------------------
# entry point
# ----------------------------------------------------------------------------

def kernel(x, edge_index, W1, b1, W2, b2, _trace=False):
    x = np.asarray(x)
    edge_index = np.asarray(edge_index)
    W1 = np.asarray(W1, dtype=np.float32)
    b1 = np.asarray(b1, dtype=np.float32)
    W2 = np.asarray(W2, dtype=np.float32)
    b2 = np.asarray(b2, dtype=np.float32)

    if "meta" not in _cache:
        _cache["meta"] = _preprocess(edge_index)
        _cache["nc"] = _build_program(_cache["meta"])
    meta = _cache["meta"]
    nc = _cache["nc"]
    order = meta["order"]

    W1r = (
        W1.reshape(4, 128, HIDDEN).transpose(1, 0, 2).reshape(128, 64).astype(
            np.float32
        )
    )
    b1r = np.tile(b1, (128, SB)).astype(np.float32)
    b2r = np.tile(b2, (128, SB)).astype(np.float32)
    f_idx = np.arange(128) % HIDDEN
    E8I = np.eye(HIDDEN, dtype=np.float32)[f_idx]          # [128, 16]
    W2r = W2[f_idx].astype(np.float32)                      # [128, 64]
    ident = np.eye(128, dtype=np.float32)
    dmask = np.ones((128, 1), dtype=np.float32)
    dmask[128 - (NPC - NPC_REAL) :] = 0.0

    in_maps = []
    for c in range(NCORES):
        lo = c * NPC_REAL
        xc = np.zeros((NPC, N_FEAT), dtype=np.float32)
        real = order[c] < NPC_REAL
        xc[real] = x[lo + order[c][real]]
        in_maps.append(
            {
                "xT": np.ascontiguousarray(xc.T),
                "idx_in": meta["idx_all"][c],
                "degrep_in": meta["deg_rep"][c],
                "W1r": W1r,
                "b1r": b1r,
                "E8I": E8I,
                "W2r": W2r,
                "b2r": b2r,
                "ident": ident,
                "dmask": dmask,
            }
        )

    res = run_bass_kernel_spmd(nc, in_maps, list(range(NCORES)), trace=_trace)
    _cache["last_res"] = res

    out = np.empty((N_NODES, N_CLASSES), dtype=np.float32)
    for c in range(NCORES):
        oc = res.results[c]["out"].reshape(NPC, N_CLASSES)  # position-major
        lo = c * NPC_REAL
        real = order[c] < NPC_REAL
        out[lo + order[c][real]] = oc[real]
    return out



# revision 4
# speedup vs baseline: 1.1683x; 1.1683x over previous
"""2-layer GCN (100k nodes, 3.2M edges) on 8 Trainium2 NeuronCores.

v2 strategy (node-partition + halo exchange, gather-optimized):
  - Nodes range-partitioned across 8 cores (12500 real + 44 dummy = 12544
    positions/core). GCN algebra: out = D^-1/2 A_hat D^-1/2 (H W); the
    src-side D^-1/2 is folded into per-edge gather masks, the dst-side is
    applied after aggregation. Self-loops are ordinary edges.
  - Per layer each core computes its 16-dim feature shard in FEATURE-MAJOR
    bf16 [16, 12544] (phase A: W1-stationary PE matmuls, no transposes),
    AllGathers shards (bf16 halves the halo traffic), then builds a PAIRED
    bf16 gather table tabB[16g+h, pos, i] = feature h of core (2*(g%4)+i)
    at pos, via PE permutation matmuls + strided DVE copies.
  - Aggregation via the ap_gather ucode in bf16 with d=2 (measured 15.1
    ns/idx vs 27.3 for f32): one index serves an edge from either core of
    the group's pair (2-choice load balancing halves slot padding); a
    per-slot bf16 mask (= dinv[src] on the matching sub-slot, 0 on the
    other) selects the real edge and applies the src-side normalization.
    A DVE segmented reduce + a PE selector matmul (sums the 8 groups)
    produce the aggregate; dst-side dinv, bias, relu / log_softmax follow
    as in the reference.

Host work is graph restructuring only (edge->slot assignment, permutations,
dinv for the masks); all x-dependent arithmetic runs on device.
"""

import numpy as np
import ml_dtypes

import concourse.bass as bass
import concourse.bacc as bacc
import concourse.mybir as mybir
import concourse.tile as tile
from concourse.bass_utils import run_bass_kernel_spmd

N_NODES = 100000
N_FEAT = 512
HIDDEN = 16
N_CLASSES = 64
NCORES = 8
NPC_REAL = 12500          # real nodes per core
NPC = 12544               # padded positions per core (98 * 128)
NBLK = NPC // 128         # 98 blocks of 128 nodes
SB = 4                    # max blocks per super-block
NI_CAP = 3072             # max slots per gather instruction
DUMMY_COL = NPC - 1       # every core's last position is a dummy (zero) node
XCHUNK = 512              # phase-A node-column chunk

_cache = {}


# ----------------------------------------------------------------------------
# host-side graph restructuring
# ----------------------------------------------------------------------------

def _preprocess(edge_index):
    src0 = edge_index[0].astype(np.int64)
    dst0 = edge_index[1].astype(np.int64)
    loop = np.arange(N_NODES, dtype=np.int64)
    src = np.concatenate([src0, loop])        # self-loops as ordinary edges
    dst = np.concatenate([dst0, loop])

    deg = np.bincount(dst, minlength=N_NODES)             # includes self-loop
    dinv = 1.0 / np.sqrt(np.maximum(deg, 1.0))

    owner = src // NPC_REAL
    pair = owner // 2                                     # 0..3
    sub = owner % 2

    # order edges by (dst, pair); split each run between groups p and p+4
    key = dst * 4 + pair
    perm = np.argsort(key, kind="stable")
    key_s = key[perm]
    src_s = src[perm]
    sub_s = sub[perm]
    pair_s = key_s % 4
    dst_s = key_s // 4
    cnt = np.bincount(key, minlength=N_NODES * 4)
    starts = np.zeros(N_NODES * 4 + 1, dtype=np.int64)
    starts[1:] = np.cumsum(cnt)
    r = np.arange(len(src_s), dtype=np.int64) - starts[key_s]
    m2 = cnt[key_s]
    half = (m2 + 1) // 2
    second = r >= half
    grp_s = pair_s + 4 * second                           # 0..7
    j_within = np.where(second, r - half, r)

    # per-node slot width: max over the 8 groups of assigned count
    halfc = (cnt.reshape(N_NODES, 4) + 1) // 2
    floorc = cnt.reshape(N_NODES, 4) // 2
    dtil = np.maximum(halfc.max(axis=1), floorc.max(axis=1))  # floor<=ceil, but keep

    # per-core permutation: sort local nodes by dtil desc; dummies last
    order = np.empty((NCORES, NPC), dtype=np.int64)   # position -> local node id
    rank = np.empty(N_NODES, dtype=np.int64)          # global node -> position
    for c in range(NCORES):
        lo = c * NPC_REAL
        d_loc = np.concatenate(
            [dtil[lo : lo + NPC_REAL], np.full(NPC - NPC_REAL, -1, np.int64)]
        )
        o = np.argsort(-d_loc, kind="stable")
        order[c] = o
        inv = np.empty(NPC, dtype=np.int64)
        inv[o] = np.arange(NPC)
        rank[lo : lo + NPC_REAL] = inv[:NPC_REAL]

    # block widths unified over cores; adaptive super-blocks capped by NI_CAP
    dtil_pos = np.zeros((NCORES, NPC), dtype=np.int64)
    for c in range(NCORES):
        lo = c * NPC_REAL
        real = order[c] < NPC_REAL
        dtil_pos[c][real] = dtil[lo + order[c][real]]
    blk_D = np.zeros(NBLK, dtype=np.int64)
    for b in range(NBLK):
        blk_D[b] = max(1, dtil_pos[:, b * 128 : (b + 1) * 128].max())
    assert 128 * blk_D.max() <= NI_CAP
    supers = []  # (b0, nblk, D)
    b = 0
    while b < NBLK:
        D = blk_D[b]
        nb = 1
        while (
            b + nb < NBLK
            and nb < SB
            and (nb + 1) * 128 * max(D, blk_D[b + nb]) <= NI_CAP
        ):
            D = max(D, blk_D[b + nb])
            nb += 1
        supers.append((b, nb, int(D)))
        b += nb
    n_super = len(supers)
    sup_of_blk = np.zeros(NBLK, dtype=np.int64)
    for si, (b0, nb, D) in enumerate(supers):
        sup_of_blk[b0 : b0 + nb] = si
    DSUP = np.array([D for (_, _, D) in supers], dtype=np.int64)
    sup_b0 = np.array([b0 for (b0, _, _) in supers], dtype=np.int64)
    num_idxs = np.array([nb * 128 * D for (_, nb, D) in supers], dtype=np.int64)
    colbase = np.zeros(n_super + 1, dtype=np.int64)
    colbase[1:] = np.cumsum(num_idxs // 16)
    IDXCOLS = int(colbase[-1])
    slotbase = np.zeros(n_super + 1, dtype=np.int64)
    slotbase[1:] = np.cumsum(num_idxs)
    TOTSLOTS = int(slotbase[-1])

    # per-edge slot assignment
    c_s = dst_s // NPC_REAL
    pos_s = rank[dst_s]
    blk_s = pos_s // 128
    i_s = pos_s % 128
    sup_s = sup_of_blk[blk_s]
    node_in_sup = (blk_s - sup_b0[sup_s]) * 128 + i_s
    e_col = node_in_sup * DSUP[sup_s] + j_within          # slot within instr
    part = 16 * grp_s + (e_col % 16)
    col = colbase[sup_s] + e_col // 16
    val = rank[src_s]

    idx_all = np.full((NCORES, 128, IDXCOLS), DUMMY_COL, dtype=np.int16)
    idx_all[c_s, part, col] = val.astype(np.int16)

    # masks: [core, 8, 2*TOTSLOTS] bf16, value dinv[src] at (grp, slot, sub)
    mask8 = np.zeros((NCORES, 8, 2 * TOTSLOTS), dtype=np.float32)
    slot_global = slotbase[sup_s] + e_col
    mask8[c_s, grp_s, 2 * slot_global + sub_s] = dinv[src_s]
    mask_all = np.repeat(mask8, 16, axis=1).astype(ml_dtypes.bfloat16)

    # dst-side dinv, node-major per core: [128, NBLK] repeated HIDDEN wide
    deg_pb = np.zeros((NCORES, 128, NBLK), dtype=np.int32)
    for c in range(NCORES):
        lo = c * NPC_REAL
        real = order[c] < NPC_REAL
        d = np.zeros(NPC, dtype=np.int32)
        d[real] = deg[lo + order[c][real]].astype(np.int32)
        deg_pb[c] = d.reshape(NBLK, 128).T
    deg_rep = np.repeat(deg_pb, HIDDEN, axis=2).reshape(NCORES, 128, NBLK * HIDDEN)

    return {
        "order": order,
        "idx_all": idx_all,
        "mask_all": mask_all,
        "deg_rep": deg_rep,
        "supers": supers,
        "num_idxs": num_idxs,
        "colbase": colbase,
        "slotbase": slotbase,
        "IDXCOLS": IDXCOLS,
        "TOTSLOTS": TOTSLOTS,
    }


# ----------------------------------------------------------------------------
# device program
# ----------------------------------------------------------------------------

def _build_program(meta):
    supers = meta["supers"]
    num_idxs = meta["num_idxs"]
    colbase = meta["colbase"]
    slotbase = meta["slotbase"]
    IDXCOLS = meta["IDXCOLS"]
    TOTSLOTS = meta["TOTSLOTS"]
    f32 = mybir.dt.float32
    bf16 = mybir.dt.bfloat16
    NXC = (NPC + XCHUNK - 1) // XCHUNK       # phase-A chunks

    nc = bacc.Bacc(
        "TRN2", target_bir_lowering=False, debug=False, num_devices=NCORES
    )
    xT = nc.declare_dram_parameter("xT", [N_FEAT, NPC], f32, isOutput=False)
    idx_in = nc.declare_dram_parameter(
        "idx_in", [128, IDXCOLS], mybir.dt.int16, isOutput=False
    )
    mask_in = nc.declare_dram_parameter(
        "mask_in", [128, 2 * TOTSLOTS], bf16, isOutput=False
    )
    degrep_in = nc.declare_dram_parameter(
        "degrep_in", [128, NBLK * HIDDEN], mybir.dt.int32, isOutput=False
    )
    W1b_in = nc.declare_dram_parameter("W1b", [128, 64], bf16, isOutput=False)
    b1r_in = nc.declare_dram_parameter("b1r", [128, SB * HIDDEN], f32, isOutput=False)
    E8I_in = nc.declare_dram_parameter("E8I", [128, HIDDEN], f32, isOutput=False)
    W2r_in = nc.declare_dram_parameter("W2r", [128, N_CLASSES], f32, isOutput=False)
    b2r_in = nc.declare_dram_parameter(
        "b2r", [128, SB * N_CLASSES], f32, isOutput=False
    )
    ident_in = nc.declare_dram_parameter("ident", [128, 128], f32, isOutput=False)
    perm_in = nc.declare_dram_parameter("perm2", [128, 256], bf16, isOutput=False)
    dmask_in = nc.declare_dram_parameter("dmask", [128, 1], f32, isOutput=False)
    out_d = nc.declare_dram_parameter("out", [NBLK, 128, N_CLASSES], f32, isOutput=True)

    q1d = nc.dram_tensor("q1d", [16, NPC], bf16)
    q2d = nc.dram_tensor("q2d", [16, NPC], bf16)
    tab1d = nc.dram_tensor("tab1d", [128, NPC], bf16, addr_space="Shared")
    tab2d = nc.dram_tensor("tab2d", [128, NPC], bf16, addr_space="Shared")

    rg = [list(range(NCORES))]

    with tile.TileContext(nc) as tc:
        with (
            tc.tile_pool(name="const", bufs=1) as cp,
            tc.tile_pool(name="xt", bufs=2) as xp,
            tc.tile_pool(name="xtb", bufs=2) as xbp,
            tc.tile_pool(name="msg", bufs=2) as mp,
            tc.tile_pool(name="maskp", bufs=2) as kp,
            tc.tile_pool(name="work", bufs=3) as wp,
            tc.tile_pool(name="shard", bufs=1) as sp,
            tc.tile_pool(name="tab", bufs=1) as tp,
            tc.tile_pool(name="ps", bufs=1, space="PSUM") as pp,
            tc.tile_pool(name="psT", bufs=2, space="PSUM") as ppT,
            tc.tile_pool(name="psO", bufs=2, space="PSUM") as ppO,
            tc.tile_pool(name="psS", bufs=2, space="PSUM") as ppS,
        ):
            # ---- constants -------------------------------------------------
            W1b = cp.tile([128, 64], bf16)
            nc.sync.dma_start(out=W1b[:], in_=W1b_in[:])
            b1r = cp.tile([128, SB * HIDDEN], f32)
            nc.sync.dma_start(out=b1r[:], in_=b1r_in[:])
            E8I = cp.tile([128, HIDDEN], f32)
            nc.sync.dma_start(out=E8I[:], in_=E8I_in[:])
            W2r = cp.tile([128, N_CLASSES], f32)
            nc.sync.dma_start(out=W2r[:], in_=W2r_in[:])
            b2r = cp.tile([128, SB * N_CLASSES], f32)
            nc.sync.dma_start(out=b2r[:], in_=b2r_in[:])
            ident = cp.tile([128, 128], f32)
            nc.sync.dma_start(out=ident[:], in_=ident_in[:])
            perm2 = cp.tile([128, 256], bf16)
            nc.sync.dma_start(out=perm2[:], in_=perm_in[:])
            dmask = cp.tile([128, 1], f32)
            nc.sync.dma_start(out=dmask[:], in_=dmask_in[:])
            idx_sb = cp.tile([128, IDXCOLS], mybir.dt.int16)
            nc.sync.dma_start(out=idx_sb[:], in_=idx_in[:])

            # dinv (dst side), node-major repeated HIDDEN-wide per block
            degrep = cp.tile([128, NBLK * HIDDEN], mybir.dt.int32)
            nc.sync.dma_start(out=degrep[:], in_=degrep_in[:])
            dinvr = cp.tile([128, NBLK * HIDDEN], f32)
            nc.vector.tensor_copy(out=dinvr[:], in_=degrep[:])
            nc.vector.tensor_scalar_max(out=dinvr[:], in0=dinvr[:], scalar1=1.0)
            nc.vector.reciprocal(out=dinvr[:], in_=dinvr[:])
            nc.scalar.activation(
                out=dinvr[:], in_=dinvr[:], func=mybir.ActivationFunctionType.Sqrt
            )

            shard = sp.tile([16, NPC], bf16)       # feature-major shard
            tab16 = tp.tile([128, NPC], bf16, tag="tab16")
            tabB = tp.tile([128, 2 * NPC], bf16, tag="tabB")

            # ---- phase A: q1 = x @ W1, feature-major shard -----------------
            for s in range(NXC):
                c0 = s * XCHUNK
                w = min(XCHUNK, NPC - c0)
                psA = pp.tile([16, XCHUNK], f32, tag="psA")
                for kc in range(4):
                    xt = xp.tile([128, XCHUNK], f32, tag="xt")
                    nc.sync.dma_start(
                        out=xt[:, :w],
                        in_=xT[kc * 128 : (kc + 1) * 128, c0 : c0 + w],
                    )
                    xtb = xbp.tile([128, XCHUNK], bf16, tag="xtb")
                    nc.scalar.activation(
                        out=xtb[:, :w], in_=xt[:, :w],
                        func=mybir.ActivationFunctionType.Copy,
                    )
                    nc.tensor.matmul(
                        out=psA[:, :w],
                        lhsT=W1b[:, kc * HIDDEN : (kc + 1) * HIDDEN],
                        rhs=xtb[:, :w],
                        start=(kc == 0),
                        stop=(kc == 3),
                    )
                nc.vector.tensor_copy(out=shard[:, c0 : c0 + w], in_=psA[:, :w])
            nc.sync.dma_start(out=q1d[:], in_=shard[:])

            # ---- table build: allgather + load + pair-shuffle --------------
            def build_table(qd, tabd):
                nc.gpsimd.collective_compute(
                    "AllGather",
                    mybir.AluOpType.bypass,
                    replica_groups=rg,
                    ins=[qd[:]],
                    outs=[tabd[:]],
                )
                nc.sync.dma_start(out=tab16[:], in_=tabd[:])
                tabBv = tabB[:].rearrange("p (n two) -> p n two", two=2)
                for s in range(NXC):
                    c0 = s * XCHUNK
                    w = min(XCHUNK, NPC - c0)
                    for i in range(2):
                        psP = ppS.tile([128, XCHUNK], f32, tag="psP")
                        nc.tensor.matmul(
                            out=psP[:, :w],
                            lhsT=perm2[:, i * 128 : (i + 1) * 128],
                            rhs=tab16[:, c0 : c0 + w],
                            start=True,
                            stop=True,
                        )
                        nc.vector.tensor_copy(
                            out=tabBv[:, c0 : c0 + w, i], in_=psP[:, :w]
                        )

            build_table(q1d, tab1d)

            # ---- aggregation helper ----------------------------------------
            def aggregate(s):
                b0, nblk_s, D = supers[s]
                nodes = nblk_s * 128
                ni = int(num_idxs[s])
                maskt = kp.tile([128, 2 * NI_CAP], bf16, tag="maskt")
                nc.sync.dma_start(
                    out=maskt[:, : 2 * ni],
                    in_=mask_in[:, 2 * int(slotbase[s]) : 2 * int(slotbase[s + 1])],
                )
                msg = mp.tile([128, 2 * NI_CAP], bf16, tag="msg")
                nc.gpsimd.ap_gather(
                    out_ap=msg[:, : 2 * ni],
                    in_ap=tabB[:],
                    idxs_ap=idx_sb[:, int(colbase[s]) : int(colbase[s + 1])],
                    channels=128,
                    num_elems=NPC,
                    d=2,
                    num_idxs=ni,
                )
                nc.vector.tensor_tensor(
                    out=msg[:, : 2 * ni],
                    in0=msg[:, : 2 * ni],
                    in1=maskt[:, : 2 * ni],
                    op=mybir.AluOpType.mult,
                )
                part = wp.tile([128, SB * 128], f32, tag="part")
                nc.vector.tensor_reduce(
                    out=part[:, :nodes],
                    in_=msg[:, : 2 * ni].rearrange("p (n dd) -> p n dd", dd=2 * D),
                    axis=mybir.AxisListType.X,
                    op=mybir.AluOpType.add,
                )
                return part, b0, nblk_s

            def post_to_shard(qa4, b0, nblk_s):
                """transpose node-major [128, nblk_s*16] -> bf16 shard strips."""
                for j in range(nblk_s):
                    b = b0 + j
                    psT = ppT.tile([HIDDEN, 128], f32, tag="psT")
                    nc.tensor.transpose(
                        out=psT[:],
                        in_=qa4[:, j * HIDDEN : (j + 1) * HIDDEN],
                        identity=ident[:],
                    )
                    nc.vector.tensor_copy(
                        out=shard[:, b * 128 : (b + 1) * 128], in_=psT[:]
                    )

            # ---- layer 1 aggregation -> q2 shard ---------------------------
            for s in range(len(supers)):
                part, b0, nblk_s = aggregate(s)
                psX = pp.tile([128, SB * HIDDEN], f32, tag="psX")
                for j in range(nblk_s):
                    nc.tensor.matmul(
                        out=psX[:, j * HIDDEN : (j + 1) * HIDDEN],
                        lhsT=part[:, j * 128 : (j + 1) * 128],
                        rhs=E8I[:],
                        start=True,
                        stop=True,
                    )
                qa4 = wp.tile([128, SB * HIDDEN], f32, tag="qa4")
                dslice = dinvr[:, b0 * HIDDEN : b0 * HIDDEN + nblk_s * HIDDEN]
                ql = qa4[:, : nblk_s * HIDDEN]
                nc.vector.tensor_tensor(
                    out=ql, in0=psX[:, : nblk_s * HIDDEN], in1=dslice,
                    op=mybir.AluOpType.mult,
                )
                nc.vector.tensor_tensor(
                    out=ql, in0=ql, in1=b1r[:, : nblk_s * HIDDEN],
                    op=mybir.AluOpType.add,
                )
                nc.vector.tensor_scalar_max(out=ql, in0=ql, scalar1=0.0)
                if b0 + nblk_s == NBLK:  # kill dummy-node tail
                    sl = qa4[:, (nblk_s - 1) * HIDDEN : nblk_s * HIDDEN]
                    nc.vector.tensor_scalar_mul(out=sl, in0=sl, scalar1=dmask[:, :1])
                post_to_shard(qa4, b0, nblk_s)
            nc.sync.dma_start(out=q2d[:], in_=shard[:])

            build_table(q2d, tab2d)

            # ---- layer 2 aggregation -> logits -> log_softmax --------------
            for s in range(len(supers)):
                part, b0, nblk_s = aggregate(s)
                psO = ppO.tile([128, SB * N_CLASSES], f32, tag="psO")
                for j in range(nblk_s):
                    nc.tensor.matmul(
                        out=psO[:, j * N_CLASSES : (j + 1) * N_CLASSES],
                        lhsT=part[:, j * 128 : (j + 1) * 128],
                        rhs=W2r[:],
                        start=True,
                        stop=True,
                    )
                z4 = wp.tile([128, SB * N_CLASSES], f32, tag="z4")
                for j in range(nblk_s):
                    b = b0 + j
                    nc.vector.tensor_scalar_mul(
                        out=z4[:, j * N_CLASSES : (j + 1) * N_CLASSES],
                        in0=psO[:, j * N_CLASSES : (j + 1) * N_CLASSES],
                        scalar1=dinvr[:, b * HIDDEN : b * HIDDEN + 1],
                    )
                zl = z4[:, : nblk_s * N_CLASSES]
                nc.vector.tensor_tensor(
                    out=zl, in0=zl, in1=b2r[:, : nblk_s * N_CLASSES],
                    op=mybir.AluOpType.add,
                )
                negm = wp.tile([128, SB], f32, tag="negm")
                nc.vector.tensor_reduce(
                    out=negm[:, :nblk_s],
                    in_=zl.rearrange("p (n c) -> p n c", c=N_CLASSES),
                    axis=mybir.AxisListType.X,
                    op=mybir.AluOpType.max,
                    negate=True,
                )
                e4 = wp.tile([128, SB * N_CLASSES], f32, tag="e4")
                ssum = wp.tile([128, SB], f32, tag="ssum")
                for j in range(nblk_s):
                    nc.scalar.activation(
                        out=e4[:, j * N_CLASSES : (j + 1) * N_CLASSES],
                        in_=z4[:, j * N_CLASSES : (j + 1) * N_CLASSES],
                        func=mybir.ActivationFunctionType.Exp,
                        bias=negm[:, j : j + 1],
                        scale=1.0,
                        accum_out=ssum[:, j : j + 1],
                    )
                ls = wp.tile([128, SB], f32, tag="ls")
                nc.scalar.activation(
                    out=ls[:, :nblk_s],
                    in_=ssum[:, :nblk_s],
                    func=mybir.ActivationFunctionType.Ln,
                )
                o4 = wp.tile([128, SB * N_CLASSES], f32, tag="o4")
                for j in range(nblk_s):
                    nc.vector.tensor_scalar(
                        out=o4[:, j * N_CLASSES : (j + 1) * N_CLASSES],
                        in0=z4[:, j * N_CLASSES : (j + 1) * N_CLASSES],
                        scalar1=negm[:, j : j + 1],
                        scalar2=ls[:, j : j + 1],
                        op0=mybir.AluOpType.add,
                        op1=mybir.AluOpType.subtract,
                    )
                for j in range(nblk_s):
                    nc.sync.dma_start(
                        out=out_d[b0 + j],
                        in_=o4[:, j * N_CLASSES : (j + 1) * N_CLASSES],
                    )

    nc.finalize()
    return nc


# ----------------------------------------------------------------------------
# entry point
# ----------------------------------------------------------------------------

def kernel(x, edge_index, W1, b1, W2, b2, _trace=False):
    x = np.asarray(x)
    edge_index = np.asarray(edge_index)
    W1 = np.asarray(W1, dtype=np.float32)
    b1 = np.asarray(b1, dtype=np.float32)
    W2 = np.asarray(W2, dtype=np.float32)
    b2 = np.asarray(b2, dtype=np.float32)

    if "meta" not in _cache:
        _cache["meta"] = _preprocess(edge_index)
        _cache["nc"] = _build_program(_cache["meta"])
    meta = _cache["meta"]
    nc = _cache["nc"]
    order = meta["order"]

    # W1 chunk-major bf16: W1b[p, kc*16+h] = W1[kc*128+p, h]
    W1b = (
        W1.reshape(4, 128, HIDDEN).transpose(1, 0, 2).reshape(128, 64)
    ).astype(ml_dtypes.bfloat16)
    b1r = np.tile(b1, (128, SB)).astype(np.float32)
    b2r = np.tile(b2, (128, SB)).astype(np.float32)
    f_idx = np.arange(128) % HIDDEN
    E8I = np.eye(HIDDEN, dtype=np.float32)[f_idx]          # [128, 16]
    W2r = W2[f_idx].astype(np.float32)                      # [128, 64]
    ident = np.eye(128, dtype=np.float32)
    dmask = np.ones((128, 1), dtype=np.float32)
    dmask[128 - (NPC - NPC_REAL) :] = 0.0
    # PERMi[p, q] = 1 iff p == 16*(2*(q//16 % 4) + i) + q%16
    perm2 = np.zeros((128, 256), dtype=np.float32)
    q = np.arange(128)
    for i in range(2):
        p_src = 16 * (2 * ((q // 16) % 4) + i) + (q % 16)
        perm2[p_src, i * 128 + q] = 1.0
    perm2 = perm2.astype(ml_dtypes.bfloat16)

    in_maps = []
    for c in range(NCORES):
        lo = c * NPC_REAL
        xc = np.zeros((NPC, N_FEAT), dtype=np.float32)
        real = order[c] < NPC_REAL
        xc[real] = x[lo + order[c][real]]
        in_maps.append(
            {
                "xT": np.ascontiguousarray(xc.T),
                "idx_in": meta["idx_all"][c],
                "mask_in": meta["mask_all"][c],
                "degrep_in": meta["deg_rep"][c],
                "W1b": W1b,
                "b1r": b1r,
                "E8I": E8I,
                "W2r": W2r,
                "b2r": b2r,
                "ident": ident,
                "perm2": perm2,
                "dmask": dmask,
            }
        )

    res = run_bass_kernel_spmd(nc, in_maps, list(range(NCORES)), trace=_trace)
    _cache["last_res"] = res

    out = np.empty((N_NODES, N_CLASSES), dtype=np.float32)
    for c in range(NCORES):
        oc = res.results[c]["out"].reshape(NPC, N_CLASSES)
        lo = c * NPC_REAL
        real = order[c] < NPC_REAL
        out[lo + order[c][real]] = oc[real]
    return out


# revision 5
# speedup vs baseline: 1.1911x; 1.0195x over previous
"""2-layer GCN on 8 NeuronCores — v4: mask-free paired bf16 gather table.

Like v2 (bf16 d=2 ap_gather, pair-of-cores groups, 2-choice balancing) but
the per-slot masks are gone: the table's position space is doubled, view
v holds (q_even[pos], 0) for v=0 and (0, q_odd[pos]) for v=1, so a slot
reads its edge's value plus an exact zero. dinv[src] is prescaled into the
shards (streamed per-chunk in phase A, node-major multiply in layer 1).
Shards/tables are staged through DRAM; no resident shard/tab16 tiles.
"""

import numpy as np
import ml_dtypes

import concourse.bass as bass
import concourse.bacc as bacc
import concourse.mybir as mybir
import concourse.tile as tile
from concourse.bass_utils import run_bass_kernel_spmd

N_NODES = 100000
N_FEAT = 512
HIDDEN = 16
N_CLASSES = 64
NCORES = 8
NPC_REAL = 12500
NPC = 12544
NBLK = NPC // 128
SB = 4
NI_CAP = 3072
DUMMY_COL = NPC - 1
XCHUNK = 512

_cache = {}


def _preprocess(edge_index):
    src0 = edge_index[0].astype(np.int64)
    dst0 = edge_index[1].astype(np.int64)
    loop = np.arange(N_NODES, dtype=np.int64)
    src = np.concatenate([src0, loop])
    dst = np.concatenate([dst0, loop])

    deg = np.bincount(dst, minlength=N_NODES)
    dinv = (1.0 / np.sqrt(np.maximum(deg, 1.0))).astype(np.float32)

    owner = src // NPC_REAL
    pair = owner // 2
    sub = owner % 2

    key = dst * 4 + pair
    perm = np.argsort(key, kind="stable")
    key_s = key[perm]
    src_s = src[perm]
    sub_s = sub[perm]
    pair_s = key_s % 4
    dst_s = key_s // 4
    cnt = np.bincount(key, minlength=N_NODES * 4)
    starts = np.zeros(N_NODES * 4 + 1, dtype=np.int64)
    starts[1:] = np.cumsum(cnt)
    r = np.arange(len(src_s), dtype=np.int64) - starts[key_s]
    m2 = cnt[key_s]
    half = (m2 + 1) // 2
    second = r >= half
    grp_s = pair_s + 4 * second
    j_within = np.where(second, r - half, r)

    halfc = (cnt.reshape(N_NODES, 4) + 1) // 2
    dtil = halfc.max(axis=1)

    order = np.empty((NCORES, NPC), dtype=np.int64)
    rank = np.empty(N_NODES, dtype=np.int64)
    for c in range(NCORES):
        lo = c * NPC_REAL
        d_loc = np.concatenate(
            [dtil[lo : lo + NPC_REAL], np.full(NPC - NPC_REAL, -1, np.int64)]
        )
        o = np.argsort(-d_loc, kind="stable")
        order[c] = o
        inv = np.empty(NPC, dtype=np.int64)
        inv[o] = np.arange(NPC)
        rank[lo : lo + NPC_REAL] = inv[:NPC_REAL]

    dtil_pos = np.zeros((NCORES, NPC), dtype=np.int64)
    for c in range(NCORES):
        lo = c * NPC_REAL
        real = order[c] < NPC_REAL
        dtil_pos[c][real] = dtil[lo + order[c][real]]
    blk_D = np.zeros(NBLK, dtype=np.int64)
    for b in range(NBLK):
        blk_D[b] = max(1, dtil_pos[:, b * 128 : (b + 1) * 128].max())
    assert 128 * blk_D.max() <= NI_CAP
    supers = []
    b = 0
    while b < NBLK:
        D = blk_D[b]
        nb = 1
        while (
            b + nb < NBLK
            and nb < SB
            and (nb + 1) * 128 * max(D, blk_D[b + nb]) <= NI_CAP
        ):
            D = max(D, blk_D[b + nb])
            nb += 1
        supers.append((b, nb, int(D)))
        b += nb
    n_super = len(supers)
    sup_of_blk = np.zeros(NBLK, dtype=np.int64)
    for si, (b0, nb, D) in enumerate(supers):
        sup_of_blk[b0 : b0 + nb] = si
    DSUP = np.array([D for (_, _, D) in supers], dtype=np.int64)
    sup_b0 = np.array([b0 for (b0, _, _) in supers], dtype=np.int64)
    num_idxs = np.array([nb * 128 * D for (_, nb, D) in supers], dtype=np.int64)
    colbase = np.zeros(n_super + 1, dtype=np.int64)
    colbase[1:] = np.cumsum(num_idxs // 16)
    IDXCOLS = int(colbase[-1])

    c_s = dst_s // NPC_REAL
    pos_s = rank[dst_s]
    blk_s = pos_s // 128
    i_s = pos_s % 128
    sup_s = sup_of_blk[blk_s]
    node_in_sup = (blk_s - sup_b0[sup_s]) * 128 + i_s
    e_col = node_in_sup * DSUP[sup_s] + j_within
    part = 16 * grp_s + (e_col % 16)
    col = colbase[sup_s] + e_col // 16
    val = rank[src_s] + NPC * sub_s          # view v = owner parity

    idx_all = np.full((NCORES, 128, IDXCOLS), DUMMY_COL, dtype=np.int16)
    idx_all[c_s, part, col] = val.astype(np.int16)

    deg_pb = np.zeros((NCORES, 128, NBLK), dtype=np.int32)
    for c in range(NCORES):
        lo = c * NPC_REAL
        real = order[c] < NPC_REAL
        d = np.zeros(NPC, dtype=np.int32)
        d[real] = deg[lo + order[c][real]].astype(np.int32)
        deg_pb[c] = d.reshape(NBLK, 128).T
    deg_rep = np.repeat(deg_pb, HIDDEN, axis=2).reshape(NCORES, 128, NBLK * HIDDEN)

    # feature-major dinv [16, NPC] per core (position order), for phase A
    dinvF = np.zeros((NCORES, 16, NPC), dtype=np.float32)
    for c in range(NCORES):
        lo = c * NPC_REAL
        real = order[c] < NPC_REAL
        dv = np.zeros(NPC, dtype=np.float32)
        dv[real] = dinv[lo + order[c][real]]
        dinvF[c] = np.broadcast_to(dv, (16, NPC))

    return {
        "order": order,
        "idx_all": idx_all,
        "deg_rep": deg_rep,
        "dinvF": dinvF,
        "supers": supers,
        "num_idxs": num_idxs,
        "colbase": colbase,
        "IDXCOLS": IDXCOLS,
    }


def _build_program(meta):
    supers = meta["supers"]
    num_idxs = meta["num_idxs"]
    colbase = meta["colbase"]
    IDXCOLS = meta["IDXCOLS"]
    f32 = mybir.dt.float32
    bf16 = mybir.dt.bfloat16
    NXC = (NPC + XCHUNK - 1) // XCHUNK

    nc = bacc.Bacc(
        "TRN2", target_bir_lowering=False, debug=False, num_devices=NCORES
    )
    xT = nc.declare_dram_parameter("xT", [N_FEAT, NPC], f32, isOutput=False)
    idx_in = nc.declare_dram_parameter(
        "idx_in", [128, IDXCOLS], mybir.dt.int16, isOutput=False
    )
    degrep_in = nc.declare_dram_parameter(
        "degrep_in", [128, NBLK * HIDDEN], mybir.dt.int32, isOutput=False
    )
    dinvF_in = nc.declare_dram_parameter("dinvF", [16, NPC], f32, isOutput=False)
    W1b_in = nc.declare_dram_parameter("W1b", [128, 64], bf16, isOutput=False)
    b1r_in = nc.declare_dram_parameter("b1r", [128, SB * HIDDEN], f32, isOutput=False)
    E8I_in = nc.declare_dram_parameter("E8I", [128, HIDDEN], f32, isOutput=False)
    W2r_in = nc.declare_dram_parameter("W2r", [128, N_CLASSES], f32, isOutput=False)
    b2r_in = nc.declare_dram_parameter(
        "b2r", [128, SB * N_CLASSES], f32, isOutput=False
    )
    ident_in = nc.declare_dram_parameter("ident", [128, 128], f32, isOutput=False)
    perm_in = nc.declare_dram_parameter("perm2", [128, 256], bf16, isOutput=False)
    dmask_in = nc.declare_dram_parameter("dmask", [128, 1], f32, isOutput=False)
    out_d = nc.declare_dram_parameter("out", [NBLK, 128, N_CLASSES], f32, isOutput=True)

    q1d = nc.dram_tensor("q1d", [16, NPC], bf16)
    q2d = nc.dram_tensor("q2d", [16, NPC], bf16)
    tab1d = nc.dram_tensor("tab1d", [128, NPC], bf16, addr_space="Shared")
    tab2d = nc.dram_tensor("tab2d", [128, NPC], bf16, addr_space="Shared")

    rg = [list(range(NCORES))]

    with tile.TileContext(nc) as tc:
        with (
            tc.tile_pool(name="const", bufs=1) as cp,
            tc.tile_pool(name="xt", bufs=2) as xp,
            tc.tile_pool(name="xtb", bufs=2) as xbp,
            tc.tile_pool(name="msg", bufs=2) as mp,
            tc.tile_pool(name="work", bufs=3) as wp,
            tc.tile_pool(name="tab", bufs=1) as tp,
            tc.tile_pool(name="ps", bufs=1, space="PSUM") as pp,
            tc.tile_pool(name="psT", bufs=2, space="PSUM") as ppT,
            tc.tile_pool(name="psO", bufs=2, space="PSUM") as ppO,
            tc.tile_pool(name="psS", bufs=2, space="PSUM") as ppS,
        ):
            W1b = cp.tile([128, 64], bf16)
            nc.sync.dma_start(out=W1b[:], in_=W1b_in[:])
            b1r = cp.tile([128, SB * HIDDEN], f32)
            nc.sync.dma_start(out=b1r[:], in_=b1r_in[:])
            E8I = cp.tile([128, HIDDEN], f32)
            nc.sync.dma_start(out=E8I[:], in_=E8I_in[:])
            W2r = cp.tile([128, N_CLASSES], f32)
            nc.sync.dma_start(out=W2r[:], in_=W2r_in[:])
            b2r = cp.tile([128, SB * N_CLASSES], f32)
            nc.sync.dma_start(out=b2r[:], in_=b2r_in[:])
            ident = cp.tile([128, 128], f32)
            nc.sync.dma_start(out=ident[:], in_=ident_in[:])
            perm2 = cp.tile([128, 256], bf16)
            nc.sync.dma_start(out=perm2[:], in_=perm_in[:])
            dmask = cp.tile([128, 1], f32)
            nc.sync.dma_start(out=dmask[:], in_=dmask_in[:])
            idx_sb = cp.tile([128, IDXCOLS], mybir.dt.int16)
            nc.sync.dma_start(out=idx_sb[:], in_=idx_in[:])

            dinvr = cp.tile([128, NBLK * HIDDEN], f32)
            nc.sync.dma_start(
                out=dinvr[:].bitcast(mybir.dt.int32), in_=degrep_in[:]
            )
            nc.vector.tensor_copy(
                out=dinvr[:], in_=dinvr[:].bitcast(mybir.dt.int32)
            )
            nc.vector.tensor_scalar_max(out=dinvr[:], in0=dinvr[:], scalar1=1.0)
            nc.vector.reciprocal(out=dinvr[:], in_=dinvr[:])
            nc.scalar.activation(
                out=dinvr[:], in_=dinvr[:], func=mybir.ActivationFunctionType.Sqrt
            )

            # paired table, doubled position space: view 0 = (q_even, 0),
            # view 1 = (0, q_odd).  Zero lanes are memset once and never
            # overwritten by the per-layer rebuilds.
            tabB = tp.tile([128, 4 * NPC], bf16, tag="tabB")
            nc.vector.memset(tabB[:], 0.0)
            tabBv = tabB[:].rearrange("p (n two) -> p n two", two=2)

            # ---- phase A: q1 = (x @ W1) * dinv -> q1d ----------------------
            for s in range(NXC):
                c0 = s * XCHUNK
                w = min(XCHUNK, NPC - c0)
                psA = pp.tile([16, XCHUNK], f32, tag="psA")
                for kc in range(4):
                    xt = xp.tile([128, XCHUNK], f32, tag="xt")
                    nc.sync.dma_start(
                        out=xt[:, :w],
                        in_=xT[kc * 128 : (kc + 1) * 128, c0 : c0 + w],
                    )
                    xtb = xbp.tile([128, XCHUNK], bf16, tag="xtb")
                    nc.scalar.activation(
                        out=xtb[:, :w], in_=xt[:, :w],
                        func=mybir.ActivationFunctionType.Copy,
                    )
                    nc.tensor.matmul(
                        out=psA[:, :w],
                        lhsT=W1b[:, kc * HIDDEN : (kc + 1) * HIDDEN],
                        rhs=xtb[:, :w],
                        start=(kc == 0),
                        stop=(kc == 3),
                    )
                dvf = wp.tile([16, XCHUNK], f32, tag="dvf")
                nc.sync.dma_start(out=dvf[:, :w], in_=dinvF_in[:, c0 : c0 + w])
                sh = wp.tile([16, XCHUNK], bf16, tag="sh")
                nc.vector.tensor_tensor(
                    out=sh[:, :w], in0=psA[:, :w], in1=dvf[:, :w],
                    op=mybir.AluOpType.mult,
                )
                nc.sync.dma_start(out=q1d[:, c0 : c0 + w], in_=sh[:, :w])

            # ---- table build: allgather + staged pair-shuffle --------------
            def build_table(qd, tabd):
                nc.gpsimd.collective_compute(
                    "AllGather",
                    mybir.AluOpType.bypass,
                    replica_groups=rg,
                    ins=[qd[:]],
                    outs=[tabd[:]],
                )
                for s in range(NXC):
                    c0 = s * XCHUNK
                    w = min(XCHUNK, NPC - c0)
                    stg = xbp.tile([128, XCHUNK], bf16, tag="stg")
                    nc.sync.dma_start(out=stg[:, :w], in_=tabd[:, c0 : c0 + w])
                    for i in range(2):
                        psP = ppS.tile([128, XCHUNK], f32, tag="psP")
                        nc.tensor.matmul(
                            out=psP[:, :w],
                            lhsT=perm2[:, i * 128 : (i + 1) * 128],
                            rhs=stg[:, :w],
                            start=True,
                            stop=True,
                        )
                        nc.vector.tensor_copy(
                            out=tabBv[:, i * NPC + c0 : i * NPC + c0 + w, i],
                            in_=psP[:, :w],
                        )

            build_table(q1d, tab1d)

            def aggregate(s):
                b0, nblk_s, D = supers[s]
                nodes = nblk_s * 128
                ni = int(num_idxs[s])
                msg = mp.tile([128, 2 * NI_CAP], bf16, tag="msg")
                nc.gpsimd.ap_gather(
                    out_ap=msg[:, : 2 * ni],
                    in_ap=tabB[:],
                    idxs_ap=idx_sb[:, int(colbase[s]) : int(colbase[s + 1])],
                    channels=128,
                    num_elems=2 * NPC,
                    d=2,
                    num_idxs=ni,
                )
                part = wp.tile([128, SB * 128], f32, tag="part")
                nc.vector.tensor_reduce(
                    out=part[:, :nodes],
                    in_=msg[:, : 2 * ni].rearrange("p (n dd) -> p n dd", dd=2 * D),
                    axis=mybir.AxisListType.X,
                    op=mybir.AluOpType.add,
                )
                return part, b0, nblk_s

            def post_to_q2d(qa4, b0, nblk_s):
                for j in range(nblk_s):
                    b = b0 + j
                    psT = ppT.tile([HIDDEN, 128], f32, tag="psT")
                    nc.tensor.transpose(
                        out=psT[:],
                        in_=qa4[:, j * HIDDEN : (j + 1) * HIDDEN],
                        identity=ident[:],
                    )
                    shb = wp.tile([16, 128], bf16, tag="shb")
                    nc.vector.tensor_copy(out=shb[:], in_=psT[:])
                    nc.sync.dma_start(
                        out=q2d[:, b * 128 : (b + 1) * 128], in_=shb[:]
                    )

            # ---- layer 1 ---------------------------------------------------
            for s in range(len(supers)):
                part, b0, nblk_s = aggregate(s)
                psX = pp.tile([128, SB * HIDDEN], f32, tag="psX")
                for j in range(nblk_s):
                    nc.tensor.matmul(
                        out=psX[:, j * HIDDEN : (j + 1) * HIDDEN],
                        lhsT=part[:, j * 128 : (j + 1) * 128],
                        rhs=E8I[:],
                        start=True,
                        stop=True,
                    )
                qa4 = wp.tile([128, SB * HIDDEN], f32, tag="qa4")
                dslice = dinvr[:, b0 * HIDDEN : b0 * HIDDEN + nblk_s * HIDDEN]
                ql = qa4[:, : nblk_s * HIDDEN]
                nc.vector.tensor_tensor(
                    out=ql, in0=psX[:, : nblk_s * HIDDEN], in1=dslice,
                    op=mybir.AluOpType.mult,
                )
                nc.vector.tensor_tensor(
                    out=ql, in0=ql, in1=b1r[:, : nblk_s * HIDDEN],
                    op=mybir.AluOpType.add,
                )
                nc.vector.tensor_scalar_max(out=ql, in0=ql, scalar1=0.0)
                nc.vector.tensor_tensor(
                    out=ql, in0=ql, in1=dslice, op=mybir.AluOpType.mult
                )
                if b0 + nblk_s == NBLK:
                    sl = qa4[:, (nblk_s - 1) * HIDDEN : nblk_s * HIDDEN]
                    nc.vector.tensor_scalar_mul(out=sl, in0=sl, scalar1=dmask[:, :1])
                post_to_q2d(qa4, b0, nblk_s)

            build_table(q2d, tab2d)

            # ---- layer 2 ---------------------------------------------------
            for s in range(len(supers)):
                part, b0, nblk_s = aggregate(s)
                psO = ppO.tile([128, SB * N_CLASSES], f32, tag="psO")
                for j in range(nblk_s):
                    nc.tensor.matmul(
                        out=psO[:, j * N_CLASSES : (j + 1) * N_CLASSES],
                        lhsT=part[:, j * 128 : (j + 1) * 128],
                        rhs=W2r[:],
                        start=True,
                        stop=True,
                    )
                z4 = wp.tile([128, SB * N_CLASSES], f32, tag="z4")
                for j in range(nblk_s):
                    b = b0 + j
                    nc.vector.tensor_scalar_mul(
                        out=z4[:, j * N_CLASSES : (j + 1) * N_CLASSES],
                        in0=psO[:, j * N_CLASSES : (j + 1) * N_CLASSES],
                        scalar1=dinvr[:, b * HIDDEN : b * HIDDEN + 1],
                    )
                zl = z4[:, : nblk_s * N_CLASSES]
                nc.vector.tensor_tensor(
                    out=zl, in0=zl, in1=b2r[:, : nblk_s * N_CLASSES],
                    op=mybir.AluOpType.add,
                )
                negm = wp.tile([128, SB], f32, tag="negm")
                nc.vector.tensor_reduce(
                    out=negm[:, :nblk_s],
                    in_=zl.rearrange("p (n c) -> p n c", c=N_CLASSES),
                    axis=mybir.AxisListType.X,
                    op=mybir.AluOpType.max,
                    negate=True,
                )
                e4 = wp.tile([128, SB * N_CLASSES], f32, tag="e4")
                ssum = wp.tile([128, SB], f32, tag="ssum")
                for j in range(nblk_s):
                    nc.scalar.activation(
                        out=e4[:, j * N_CLASSES : (j + 1) * N_CLASSES],
                        in_=z4[:, j * N_CLASSES : (j + 1) * N_CLASSES],
                        func=mybir.ActivationFunctionType.Exp,
                        bias=negm[:, j : j + 1],
                        scale=1.0,
                        accum_out=ssum[:, j : j + 1],
                    )
                ls = wp.tile([128, SB], f32, tag="ls")
                nc.scalar.activation(
                    out=ls[:, :nblk_s],
                    in_=ssum[:, :nblk_s],
                    func=mybir.ActivationFunctionType.Ln,
                )
                o4 = wp.tile([128, SB * N_CLASSES], f32, tag="o4")
                for j in range(nblk_s):
                    nc.vector.tensor_scalar(
                        out=o4[:, j * N_CLASSES : (j + 1) * N_CLASSES],
                        in0=z4[:, j * N_CLASSES : (j + 1) * N_CLASSES],
                        scalar1=negm[:, j : j + 1],
                        scalar2=ls[:, j : j + 1],
                        op0=mybir.AluOpType.add,
                        op1=mybir.AluOpType.subtract,
                    )
                for j in range(nblk_s):
                    nc.sync.dma_start(
                        out=out_d[b0 + j],
                        in_=o4[:, j * N_CLASSES : (j + 1) * N_CLASSES],
                    )

    nc.finalize()
    return nc


def kernel(x, edge_index, W1, b1, W2, b2, _trace=False):
    x = np.asarray(x)
    edge_index = np.asarray(edge_index)
    W1 = np.asarray(W1, dtype=np.float32)
    b1 = np.asarray(b1, dtype=np.float32)
    W2 = np.asarray(W2, dtype=np.float32)
    b2 = np.asarray(b2, dtype=np.float32)

    if "meta" not in _cache:
        _cache["meta"] = _preprocess(edge_index)
        _cache["nc"] = _build_program(_cache["meta"])
    meta = _cache["meta"]
    nc = _cache["nc"]
    order = meta["order"]

    W1b = (
        W1.reshape(4, 128, HIDDEN).transpose(1, 0, 2).reshape(128, 64)
    ).astype(ml_dtypes.bfloat16)
    b1r = np.tile(b1, (128, SB)).astype(np.float32)
    b2r = np.tile(b2, (128, SB)).astype(np.float32)
    f_idx = np.arange(128) % HIDDEN
    E8I = np.eye(HIDDEN, dtype=np.float32)[f_idx]
    W2r = W2[f_idx].astype(np.float32)
    ident = np.eye(128, dtype=np.float32)
    dmask = np.ones((128, 1), dtype=np.float32)
    dmask[128 - (NPC - NPC_REAL) :] = 0.0
    perm2 = np.zeros((128, 256), dtype=np.float32)
    q = np.arange(128)
    for i in range(2):
        p_src = 16 * (2 * ((q // 16) % 4) + i) + (q % 16)
        perm2[p_src, i * 128 + q] = 1.0
    perm2 = perm2.astype(ml_dtypes.bfloat16)

    in_maps = []
    for c in range(NCORES):
        lo = c * NPC_REAL
        xc = np.zeros((NPC, N_FEAT), dtype=np.float32)
        real = order[c] < NPC_REAL
        xc[real] = x[lo + order[c][real]]
        in_maps.append(
            {
                "xT": np.ascontiguousarray(xc.T),
                "idx_in": meta["idx_all"][c],
                "degrep_in": meta["deg_rep"][c],
                "dinvF": meta["dinvF"][c],
                "W1b": W1b,
                "b1r": b1r,
                "E8I": E8I,
                "W2r": W2r,
                "b2r": b2r,
                "ident": ident,
                "perm2": perm2,
                "dmask": dmask,
            }
        )

    res = run_bass_kernel_spmd(nc, in_maps, list(range(NCORES)), trace=_trace)
    _cache["last_res"] = res

    out = np.empty((N_NODES, N_CLASSES), dtype=np.float32)
    for c in range(NCORES):
        oc = res.results[c]["out"].reshape(NPC, N_CLASSES)
        lo = c * NPC_REAL
        real = order[c] < NPC_REAL
        out[lo + order[c][real]] = oc[real]
    return out
